# revision 26
# baseline (speedup 1.0000x reference)
"""Trainium2 Bass kernel for nn_AttentionBlock (B=2, C=256, D=H=W=16).

Pipeline: GroupNorm(8) -> 1x1x1 conv QKV -> single-head attention over
N=4096 spatial tokens -> 1x1x1 conv proj -> residual.

Sharding: 8 cores = 2 batches x 4 query-chunks of 1024 tokens.  Each core
computes group-norm stats + K/V' for its full batch (redundantly across the
4 cores sharing a batch) and attention only for its 1024 queries.  Tokens
are permuted per-core (host side) so the core's own queries are tokens
0..1023 of its xb buffer; attention is permutation-invariant over keys, so
K/V/stats over the permuted order are exact.

Changes vs the 108us baseline (measured 72.2us, rel err 2.7e-3):
  - x shipped once as bf16 [2 halves, 128, 4096] (2 MB vs 5 MB of f32
    xb+xq): stats, K/V/Q matmuls, and the residual all read it; weights
    shipped bf16.  Tokens permuted per-core so own queries are tokens
    0..1023 (kills the separate xq load).
  - attention runs in fp8e4m3 with DoubleRow (256-deep contraction in one
    matmul at 0.5 cyc/row): Q/K packed [128, 2(half), n], V'^T packed as
    j-pair tiles [128, 16, 2, 272] (256 ch | ones | pad to a 16-aligned
    stride); probabilities exp(s-4) written fp8 (shift keeps exp under the
    448 fp8 max and cancels in the softmax normalize).  The 1/sqrt(C)
    score scale is split 1/4 into each of wq, wk so fp8 q/k values sit in
    e4m3's comfortable range.
  - group stats: DVE bn_stats over all 8 chunks per half (during the DMA),
    then one batched [128,2]-wide chain on DVE; the 32-channel group
    reduction uses the 32x32 block-transpose trick.
  - two-stage software pipeline across in-NEFF reps (bufs=2 tile pools):
    stage_prep (loads + stats + folds; no PE/ACT work) for rep k+1 is
    emitted mid-attention of rep k, so the reps-slope steady state hides
    the whole prologue.  Loads ride the ACT HWDGE queue and stores the SP
    HWDGE queue: a DMA trigger blocks its issuing sequencer until data is
    ready, so loads and stores must not share a queue with each other.

Layouts: scores are computed transposed (keys on partitions, queries on the
free axis) so the probability tiles are directly usable as the stationary
operand of the out-matmul; a ones-column in V'^T accumulates the softmax
denominator; outputs arrive [query, channel] and are transposed
back 128x128 via the PE transpose path.
"""

import os
import sys

import numpy as np

if "/opt/trn_rl_repo" not in sys.path:
    sys.path.insert(0, "/opt/trn_rl_repo")

import concourse.bass as bass
import concourse.mybir as mybir
import concourse.tile as tile
from concourse.bass_utils import run_bass_kernel_spmd

F32 = mybir.dt.float32
F32R = mybir.dt.float32r
BF16 = mybir.dt.bfloat16
FP8 = mybir.dt.float8e4
I32 = mybir.dt.int32
AF = mybir.ActivationFunctionType
ALU = mybir.AluOpType

B = 2
C = 256
N = 4096          # D*H*W tokens
NQ = 1024         # queries per core
G = 8             # groupnorm groups
GS = C // G       # 32 channels per group
EPS = 1e-5
NCORES = 8

SPLIT = True  # apply split_waits (needed for walrus)

_WS_CTR = [0]


def split_waits(nc, cap=1):
    """walrus allows a single sync wait per instruction; move excess
    sync_info.on_wait entries onto same-engine NoOps inserted before."""
    for fn in nc.m.functions:
        for blk in fn.blocks:
            out = []
            changed = False
            for ins in blk.instructions:
                si = ins.sync_info
                waits = list(si.on_wait) if si is not None else []
                if len(waits) > cap:
                    for i in range(0, len(waits) - cap, cap):
                        nop = mybir.InstNoOp(
                            name=f"I-waitsplit-{_WS_CTR[0]}",
                            engine=ins.engine,
                            ins=[], outs=[],
                        )
                        nop.sync_info = mybir.SyncInfo(
                            on_wait=waits[i:i + cap], on_update=[]
                        )
                        _WS_CTR[0] += 1
                        out.append(nop)
                    ins.sync_info = mybir.SyncInfo(
                        on_wait=waits[len(waits) - cap:],
                        on_update=list(si.on_update),
                    )
                    changed = True
                out.append(ins)
            if changed:
                blk.instructions = out


def build_bass(reps=1):
    nc = bass.Bass(trn_type="TRN2")

    # ---- DRAM I/O ----
    xb_d = nc.dram_tensor("xb", [2, 128, N], BF16, kind="ExternalInput")
    # f32r consts: blockdiag(1/32) 128 | identity 128 | scb 8
    cstf_d = nc.dram_tensor("cstf", [128, 264], F32R, kind="ExternalInput")
    # bf16 consts: wqk0 512 | wpv0 256 | wqk1 512 | wpv1 256
    cstb_d = nc.dram_tensor("cstb", [128, 1536], BF16, kind="ExternalInput")
    out_d = nc.dram_tensor("out", [2, 128, NQ], F32, kind="ExternalOutput")

    with tile.TileContext(nc) as tc:
        with (
            tc.tile_pool(name="consts", bufs=2) as consts,
            tc.tile_pool(name="fixed", bufs=1) as fixed,
            tc.tile_pool(name="work", bufs=4) as work,
            tc.tile_pool(name="small", bufs=4) as small,
            tc.tile_pool(name="pmisc", bufs=4, space="PSUM") as pmisc,
            tc.tile_pool(name="psO", bufs=1, space="PSUM") as psO,
        ):
            fx = {}  # rep-0-only fixed tiles (exp table, int consts, ones)

            def stage_prep(rep):
                """Input DMAs (SP queue), DVE group stats + scale/shift chain,
                weight folds.  No PE, no ACT: safe to emit mid-attention of
                the previous rep."""
                S = {}
                cstf = consts.tile([128, 264], F32R, tag="cstf")
                nc.scalar.dma_start(out=cstf, in_=cstf_d[:])
                cstb = consts.tile([128, 1536], BF16, tag="cstb")
                nc.scalar.dma_start(out=cstb, in_=cstb_d[:])
                S["ident"] = cstf[:, 128:256]
                scb = cstf.bitcast(F32)[:, 256:264]
                S["scb"] = scb
                S["wqk"] = [cstb[:, 0:512], cstb[:, 768:1280]]
                S["wpv"] = [cstb[:, 512:768], cstb[:, 1280:1536]]

                if rep == 0:
                    # exp ACT table preload + int consts + ones (once)
                    wtab = fixed.tile([128, 1], F32, tag="wtab")
                    nc.vector.memset(wtab, 0.0)
                    nc.scalar.activation(out=wtab, in_=wtab, func=AF.Exp)
                    fx["magic"] = fixed.tile([128, 2, 1], I32, tag="magic", name="magic")
                    nc.vector.memset(fx["magic"], 0x5F3759DF)
                    fx["sh1"] = fixed.tile([128, 2, 1], I32, tag="sh1", name="sh1")
                    nc.vector.memset(fx["sh1"], 1)
                    fx["ones"] = fixed.tile([128, 2], F32, tag="ones", name="ones")
                    nc.vector.memset(fx["ones"][:, 0:1], 1.0)
                    nc.vector.memset(fx["ones"][:, 1:2], 0.0)
                    fx["negs"] = fixed.tile([128, 1], F32, tag="negs", name="negs")
                    nc.vector.memset(fx["negs"], -4.0)
                    # PE HAM warm-up while DMAs run (rep 0 only)
                    wps = pmisc.tile([128, 512], F32, tag="pmisc",
                                     name="warmps")
                    for w in range(8):
                        nc.tensor.matmul(
                            wps[:, 0:128], lhsT=S["ident"], rhs=S["ident"],
                            start=True, stop=True, skip_group_check=True,
                        )

                xb = [consts.tile([128, N], BF16, tag=f"xb{t}",
                                  name=f"xb{t}_{rep}") for t in range(2)]
                S["xb"] = xb
                sts = [small.tile([128, 8, 6], F32, tag=f"bnst{t}",
                                  name=f"bnst{t}_{rep}") for t in range(2)]
                for t in range(2):
                    xrc = xb[t].rearrange("p (s c) -> p s c", c=512)
                    for ch in range(4):
                        nc.scalar.dma_start(
                            out=xb[t][:, ch * 1024:(ch + 1) * 1024],
                            in_=xb_d[t, :, ch * 1024:(ch + 1) * 1024],
                        )
                        for i in (2 * ch, 2 * ch + 1):
                            nc.vector.bn_stats(
                                out=sts[t][:, i, :], in_=xrc[:, i, :]
                            )
                        if rep == 0:
                            wps2 = pmisc.tile([128, 512], F32, tag="pmisc",
                                              name=f"warm{t}_{ch}")
                            nc.tensor.matmul(
                                wps2[:, 0:128], lhsT=S["ident"],
                                rhs=S["ident"],
                                start=True, stop=True, skip_group_check=True,
                            )

                # batched stats chain, all on DVE, both halves at once
                MV = small.tile([128, 2, 2], F32, tag="mv")     # [E, V] per t
                for t in range(2):
                    nc.vector.bn_aggr(out=MV[:, t, :], in_=sts[t])
                sq = small.tile([128, 2, 1], F32, tag="sq")
                nc.vector.tensor_tensor(sq, MV[:, :, 0:1], MV[:, :, 0:1],
                                        op=ALU.mult)
                Pr = small.tile([128, 2, 2], F32, tag="pr")     # [E, M2]
                nc.vector.tensor_tensor(Pr[:, :, 1:2], MV[:, :, 1:2], sq,
                                        op=ALU.add)
                nc.vector.tensor_copy(Pr[:, :, 0:1], MV[:, :, 0:1])
                # group reduce: broadcast cols to 32-blocks, 32x32 block
                # transpose, free-axis reduce => per-group sums on DVE
                pp = small.tile([128, 4, GS], F32, tag="pp")
                Prf = Pr.rearrange("p a b -> p (a b)")
                nc.vector.tensor_copy(
                    pp, Prf[:, :, None].to_broadcast([128, 4, GS])
                )
                tr = small.tile([128, 4, GS], F32, tag="tr")
                nc.vector.transpose(tr.rearrange("p a b -> p (a b)"),
                                    pp.rearrange("p a b -> p (a b)"))
                red = small.tile([128, 4], F32, tag="red")
                nc.vector.reduce_sum(red, tr, axis=mybir.AxisListType.X)
                nc.vector.tensor_scalar(red, red, 1.0 / GS, None, ALU.mult)
                red2 = red.rearrange("p (a b) -> p a b", b=2)  # [Eg, M2g]
                gsq = small.tile([128, 2, 1], F32, tag="gsq")
                nc.vector.tensor_tensor(gsq, red2[:, :, 0:1], red2[:, :, 0:1],
                                        op=ALU.mult)
                gv = small.tile([128, 2, 1], F32, tag="gv")
                nc.vector.scalar_tensor_tensor(
                    gv, red2[:, :, 1:2], EPS, gsq, op0=ALU.add,
                    op1=ALU.subtract,
                )
                # rstd = rsqrt(gv): bit-trick seed + 2 Newton steps
                yb = small.tile([128, 2, 1], I32, tag="yb")
                nc.vector.tensor_tensor(yb, gv.bitcast(I32), fx["sh1"],
                                        op=ALU.logical_shift_right)
                nc.vector.tensor_tensor(yb, fx["magic"], yb, op=ALU.subtract)
                y = yb.bitcast(F32)
                t2 = small.tile([128, 2, 1], F32, tag="t2")
                for _ in range(2):
                    nc.vector.tensor_tensor(t2, y, y, op=ALU.mult)
                    nc.vector.tensor_tensor(t2, t2, gv, op=ALU.mult)
                    nc.vector.tensor_scalar(t2, t2, -0.5, 1.5,
                                            ALU.mult, ALU.add)
                    nc.vector.tensor_tensor(y, y, t2, op=ALU.mult)
                # scale = rstd * norm_w ; shift = norm_b - mean*scale
                sc = small.tile([128, 2, 1], F32, tag="sc")
                nwv = scb[:, 4:6].rearrange("p (a b) -> p a b", b=1)
                nbv = scb[:, 6:8].rearrange("p (a b) -> p a b", b=1)
                nc.vector.tensor_tensor(sc, y, nwv, op=ALU.mult)
                sh = small.tile([128, 2, 1], F32, tag="shv")
                nc.vector.tensor_tensor(sh, red2[:, :, 0:1], sc, op=ALU.mult)
                nc.vector.tensor_tensor(sh, nbv, sh, op=ALU.subtract)
                shb = small.tile([128, 2, 1], BF16, tag="shb")
                nc.vector.tensor_copy(shb, sh)
                S["shb"] = shb

                # fold scale into weights (bf16)
                S["wqk_s"] = []
                S["wpv_s"] = []
                for t in range(2):
                    ws = consts.tile([128, 512], BF16, tag=f"wqks{t}",
                                     name=f"wqks{t}_{rep}")
                    nc.vector.tensor_scalar_mul(ws, S["wqk"][t],
                                                sc[:, t:t + 1, 0])
                    S["wqk_s"].append(ws)
                    ws2 = consts.tile([128, 256], BF16, tag=f"wpvs{t}",
                                      name=f"wpvs{t}_{rep}")
                    nc.vector.tensor_scalar_mul(ws2, S["wpv"][t],
                                                sc[:, t:t + 1, 0])
                    S["wpv_s"].append(ws2)
                return S

            def stage_main(rep, S, midhook):
                """K/V/Q + attention + epilogue + stores (SP queue)."""
                xb = S["xb"]
                wqk_s, wpv_s = S["wqk_s"], S["wpv_s"]
                ident = S["ident"]
                scb = S["scb"]

                # K packed fp8 [128, 2(m), N] for DoubleRow scores
                Kp = consts.tile([128, 2, N], FP8, tag="Kp", name=f"Kp_{rep}")
                for m in range(2):
                    for ch in range(N // 512):
                        ps = pmisc.tile([128, 512], F32, tag="pmisc")
                        for t in range(2):
                            nc.tensor.matmul(
                                ps,
                                lhsT=wqk_s[t][:, (2 + m) * 128:(3 + m) * 128],
                                rhs=xb[t][:, ch * 512:(ch + 1) * 512],
                                start=(t == 0),
                                stop=(t == 1),
                            )
                        if (m * (N // 512) + ch) % 2 == 0:
                            nc.scalar.activation(
                                out=Kp[:, m, ch * 512:(ch + 1) * 512], in_=ps,
                                func=AF.Copy,
                            )
                        else:
                            nc.vector.tensor_copy(
                                Kp[:, m, ch * 512:(ch + 1) * 512], ps
                            )

                # V'^T j-pair tiles [jj][o][256 ch | ones | pad to 272], fp8
                VT = consts.tile([128, 16, 2, 272], FP8, tag="VT",
                                 name=f"VT_{rep}")
                nc.vector.memset(VT[:, :, :, 256:272], 0.0)
                nc.vector.tensor_copy(
                    VT[:, :, :, 256:257],
                    fx["ones"][:, None, None, 0:1].to_broadcast([128, 16, 2, 1]),
                )
                for j in range(32):
                    ps = pmisc.tile([128, 512], F32, tag="pmisc")
                    for t in range(2):
                        nc.tensor.matmul(
                            ps[:, 0:256],
                            lhsT=xb[t][:, j * 128:(j + 1) * 128],
                            rhs=wpv_s[t],
                            start=(t == 0),
                            stop=(t == 1),
                        )
                    nc.vector.tensor_copy(
                        VT[:, j // 2, j % 2, 0:256], ps[:, 0:256]
                    )

                # shift-induced biases
                qbias = []
                fbias = []
                for m in range(2):
                    ps = pmisc.tile([128, 512], F32, tag="pmisc")
                    for t in range(2):
                        nc.tensor.matmul(
                            ps[:, 0:1],
                            lhsT=S["wqk"][t][:, m * 128:(m + 1) * 128],
                            rhs=S["shb"][:, t, :],
                            start=(t == 0),
                            stop=(t == 1),
                        )
                    qb_m = small.tile([128, 1], F32, tag=f"qbias{m}",
                                      name=f"qbias{m}_{rep}")
                    nc.vector.tensor_add(qb_m, ps[:, 0:1], scb[:, m:m + 1])
                    qbias.append(qb_m)
                for m in range(2):
                    ps = pmisc.tile([128, 512], F32, tag="pmisc")
                    for t in range(2):
                        nc.tensor.matmul(
                            ps[:, 0:1],
                            lhsT=S["wpv"][t][:, m * 128:(m + 1) * 128],
                            rhs=S["shb"][:, t, :],
                            start=(t == 0),
                            stop=(t == 1),
                        )
                    fb_m = small.tile([128, 1], F32, tag=f"fbias{m}",
                                      name=f"fbias{m}_{rep}")
                    nc.vector.tensor_add(fb_m, ps[:, 0:1],
                                         scb[:, 2 + m:3 + m])
                    fbias.append(fb_m)

                # Q packed fp8 [128, 2(m), NQ] (own tokens are xb[:, 0:1024])
                Qp = consts.tile([128, 2, NQ], FP8, tag="Qp", name=f"Qp_{rep}")
                for m in range(2):
                    for ch in range(NQ // 512):
                        ps = pmisc.tile([128, 512], F32, tag="pmisc")
                        for t in range(2):
                            nc.tensor.matmul(
                                ps,
                                lhsT=wqk_s[t][:, m * 128:(m + 1) * 128],
                                rhs=xb[t][:, ch * 512:(ch + 1) * 512],
                                start=(t == 0),
                                stop=(t == 1),
                            )
                        nc.vector.tensor_scalar_add(
                            Qp[:, m, ch * 512:(ch + 1) * 512], ps, qbias[m]
                        )

                # attention (scores and out both fp8 DoubleRow, 256-deep)
                fin = [consts.tile([128, NQ], F32, tag=f"fin{m}",
                                   name=f"fin{m}_{rep}") for m in range(2)]
                pes = {}

                def emit_scores(qt, jj):
                    """Scores + exp for the j-pair (2*jj, 2*jj+1): two
                    DoubleRow matmuls, two exps into one packed pe2 tile."""
                    pe2 = work.tile([128, 2, 512], FP8, tag="pexp",
                                    name=f"pe{rep}_{qt}_{jj}")
                    for o in range(2):
                        j = 2 * jj + o
                        ss = pmisc.tile([128, 512], F32, tag="pmisc",
                                        name=f"ss{rep}_{qt}_{j}")
                        nc.tensor.matmul(
                            ss,
                            lhsT=Kp[:, :, j * 128:(j + 1) * 128],
                            rhs=Qp[:, :, qt * 512:(qt + 1) * 512],
                            start=True, stop=True,
                            perf_mode=mybir.MatmulPerfMode.DoubleRow,
                        )
                        # exp(s - 4): keeps exp values inside fp8e4m3 range
                        # (max score ~6.2 -> exp 473 > 448 would NaN); the
                        # constant shift cancels in the softmax normalize
                        nc.scalar.activation(out=pe2[:, o, :], in_=ss,
                                             func=AF.Exp, bias=fx["negs"])
                    return pe2

                ret = None
                for qt in range(NQ // 512):
                    po = [psO.tile([128, 272], F32, tag=f"po{qs}",
                                   name=f"po{rep}_{qt}_{qs}")
                          for qs in range(4)]
                    if qt == 0:
                        pes[(0, 0)] = emit_scores(0, 0)
                    for jj in range(16):
                        pe2 = pes.pop((qt, jj))
                        for qs in range(4):
                            nc.tensor.matmul(
                                po[qs],
                                lhsT=pe2[:, :, qs * 128:(qs + 1) * 128],
                                rhs=VT[:, jj, :, :],
                                start=(jj == 0),
                                stop=(jj == 15),
                                perf_mode=mybir.MatmulPerfMode.DoubleRow,
                            )
                        if jj + 1 < 16:
                            pes[(qt, jj + 1)] = emit_scores(qt, jj + 1)
                        elif qt == 0:
                            pes[(1, 0)] = emit_scores(1, 0)
                    if qt == 0 and midhook is not None:
                        # emit the next rep's prep stage here: its DMA/DVE
                        # work overlaps this rep's second attention tile
                        ret = midhook()
                    # normalize, transpose to [channel, query], bias, residual.
                    # All DVE normalizes first, then the PE transposes, so the
                    # PE is not gated on an interleaved per-qs DVE chain.
                    aos = []
                    for qs in range(4):
                        zr = small.tile([128, 1], F32, tag="zr")
                        nc.vector.reciprocal(zr, po[qs][:, 256:257])
                        ao = work.tile([128, 256], F32R, tag="ao")
                        nc.vector.tensor_scalar_mul(ao, po[qs][:, 0:256], zr)
                        aos.append(ao)
                    for qs in range(4):
                        col = (qt * 4 + qs) * 128
                        for m in range(2):
                            tp = pmisc.tile([128, 512], F32, tag="pmisc")
                            nc.tensor.transpose(
                                tp.bitcast(F32R)[:, 0:128],
                                aos[qs][:, m * 128:(m + 1) * 128], ident
                            )
                            nc.vector.tensor_scalar_add(
                                fin[m][:, col:col + 128],
                                tp.bitcast(F32R)[:, 0:128],
                                fbias[m],
                            )
                            nc.vector.tensor_add(
                                fin[m][:, col:col + 128],
                                fin[m][:, col:col + 128],
                                xb[m][:, col:col + 128],
                            )
                        # store per 256-col half as soon as it's complete
                        # (SP HWDGE queue: its sequencer is idle, so the
                        # trigger's data-wait blocks nothing)
                        if qs % 2 == 1:
                            cs = (qt * 4 + qs - 1) * 128
                            for m in range(2):
                                nc.sync.dma_start(
                                    out=out_d[m, :, cs:cs + 256],
                                    in_=fin[m][:, cs:cs + 256],
                                )
                return ret

            S = stage_prep(0)
            for rep in range(reps):
                if rep + 1 < reps:
                    nextS = stage_main(
                        rep, S, midhook=lambda r=rep: stage_prep(r + 1))
                    S = nextS
                else:
                    stage_main(rep, S, midhook=None)

    if SPLIT:
        split_waits(nc)
    return nc


_CACHED = {}
_RUNNER = {}


def _get_nc(reps=1):
    if reps not in _CACHED:
        _CACHED[reps] = build_bass(reps)
    return _CACHED[reps]


def _get_runner(reps=1):
    """Cached jitted shard_map runner over 8 cores."""
    if reps in _RUNNER:
        return _RUNNER[reps]
    import jax
    from jax.experimental.shard_map import shard_map
    from jax.sharding import Mesh, PartitionSpec
    from concourse import bass2jax, mybir as mb
    from concourse.bass2jax import _bass_exec_p, install_neuronx_cc_hook

    nc = _get_nc(reps)
    install_neuronx_cc_hook()
    assert nc.dbg_addr is None
    partition_name = nc.partition_id_tensor.name if nc.partition_id_tensor else None

    in_names = []
    out_names = []
    out_avals = []
    zero_outs = []
    for alloc in nc.m.functions[0].allocations:
        if not isinstance(alloc, mb.MemoryLocationSet):
            continue
        name = alloc.memorylocations[0].name
        if alloc.kind == "ExternalInput":
            if name != partition_name:
                in_names.append(name)
        elif alloc.kind == "ExternalOutput":
            out_names.append(name)
            shape = tuple(alloc.tensor_shape)
            dtype = mb.dt.np(alloc.dtype)
            out_avals.append(jax.core.ShapedArray(shape, dtype))
            zero_outs.append(np.zeros(shape, dtype))
    n_params = len(in_names)
    all_in_names = in_names + out_names
    if partition_name is not None:
        all_in_names = all_in_names + [partition_name]

    def _body(*args):
        operands = list(args)
        if partition_name is not None:
            operands.append(bass2jax.partition_id_tensor())
        outs = _bass_exec_p.bind(
            *operands,
            out_avals=tuple(out_avals),
            in_names=tuple(all_in_names),
            out_names=tuple(out_names),
            lowering_input_output_aliases=(),
            sim_require_finite=True,
            sim_require_nnan=True,
            nc=nc,
        )
        return tuple(outs)

    devices = jax.devices()[:NCORES]
    mesh = Mesh(np.asarray(devices), ("core",))
    n_outs = len(out_names)
    sharded = jax.jit(
        shard_map(
            _body,
            mesh=mesh,
            in_specs=(PartitionSpec("core"),) * (n_params + n_outs),
            out_specs=(PartitionSpec("core"),) * n_outs,
            check_rep=False,
        ),
        keep_unused=True,
    )
    _RUNNER[reps] = (sharded, in_names, out_names, out_avals, zero_outs, mesh)
    return _RUNNER[reps]


def _concat_inputs(in_maps, in_names, zero_outs):
    concat_in = [
        np.concatenate([np.asarray(in_maps[c][name]) for c in range(NCORES)], axis=0)
        for name in in_names
    ]
    concat_zeros = [
        np.zeros((NCORES * z.shape[0], *z.shape[1:]), z.dtype) for z in zero_outs
    ]
    return concat_in, concat_zeros


def _run(in_maps):
    sharded, in_names, out_names, out_avals, zero_outs, mesh = _get_runner()
    concat_in, concat_zeros = _concat_inputs(in_maps, in_names, zero_outs)
    out_arrs = sharded(*concat_in, *concat_zeros)
    return [
        {
            name: np.asarray(out_arrs[i]).reshape(NCORES, *out_avals[i].shape)[c]
            for i, name in enumerate(out_names)
        }
        for c in range(NCORES)
    ]


def _to_bf16(a):
    import ml_dtypes
    return np.ascontiguousarray(
        np.asarray(a, dtype=np.float32).astype(ml_dtypes.bfloat16)
    )


def _host_prep(x, norm_w, norm_b, qkv_w, qkv_b, proj_w, proj_b):
    # split the 1/sqrt(C)=1/16 score scale as 1/4 into each of wq, wk so
    # the fp8-stored q and k values sit in e4m3's comfortable range
    wq = qkv_w[0:C] * np.float32(0.25)
    wk = qkv_w[C:2 * C] * np.float32(0.25)
    wv = qkv_w[2 * C:3 * C]
    wqkT = np.ascontiguousarray(
        np.concatenate([wq, wk], axis=0).T
    ).reshape(2, 128, 512)
    wpvT = np.ascontiguousarray((proj_w @ wv).T).reshape(2, 128, 256)
    qb = (qkv_b[0:C] * np.float32(0.25)).reshape(2, 128).T  # [128, 2]
    cb = (proj_w @ qkv_b[2 * C:3 * C] + proj_b).reshape(2, 128).T
    nw = norm_w.reshape(2, 128).T
    nb = norm_b.reshape(2, 128).T
    scb = np.concatenate([qb, cb, nw, nb], axis=1).astype(np.float32)
    ident = np.eye(128, dtype=np.float32)
    bd = np.kron(np.eye(4, dtype=np.float32),
                 np.full((GS, GS), 1.0 / GS, dtype=np.float32))
    cstf = np.ascontiguousarray(
        np.concatenate([bd, ident, scb], axis=1)
    ).astype(np.float32)
    cstb = _to_bf16(np.concatenate(
        [wqkT[0], wpvT[0], wqkT[1], wpvT[1]], axis=1
    ))

    xf = x.reshape(B, 2, 128, N)
    in_maps = []
    for core in range(NCORES):
        b, qi = divmod(core, NCORES // B)
        # permute tokens: own 1024 queries first
        perm = np.concatenate([
            np.arange(qi * NQ, (qi + 1) * NQ),
            np.arange(0, qi * NQ),
            np.arange((qi + 1) * NQ, N),
        ])
        in_maps.append(
            {
                "xb": _to_bf16(xf[b][:, :, perm]),
                "cstf": cstf,
                "cstb": cstb,
            }
        )
    return in_maps


def kernel(x, norm_w, norm_b, qkv_w, qkv_b, proj_w, proj_b):
    x = np.ascontiguousarray(np.asarray(x, dtype=np.float32))
    norm_w = np.asarray(norm_w, dtype=np.float32)
    norm_b = np.asarray(norm_b, dtype=np.float32)
    qkv_w = np.asarray(qkv_w, dtype=np.float32)
    qkv_b = np.asarray(qkv_b, dtype=np.float32)
    proj_w = np.asarray(proj_w, dtype=np.float32)
    proj_b = np.asarray(proj_b, dtype=np.float32)

    Bs, Cs = x.shape[0], x.shape[1]
    assert (Bs, Cs) == (B, C) and x.shape[2] * x.shape[3] * x.shape[4] == N

    in_maps = _host_prep(x, norm_w, norm_b, qkv_w, qkv_b, proj_w, proj_b)
    results = _run(in_maps)

    y = np.empty((B, C, N), dtype=np.float32)
    for core in range(NCORES):
        b, qi = divmod(core, NCORES // B)
        y[b, :, qi * NQ:(qi + 1) * NQ] = results[core]["out"].reshape(C, NQ)
    return y.reshape(x.shape)


def bench(in_maps, iters=50, warmup=3, reps=1):
    """Amortized per-execution device time."""
    import time
    import jax
    from jax.sharding import NamedSharding, PartitionSpec

    sharded, in_names, out_names, out_avals, zero_outs, mesh = _get_runner(reps)
    concat_in, concat_zeros = _concat_inputs(in_maps, in_names, zero_outs)
    sh = NamedSharding(mesh, PartitionSpec("core"))
    dev_in = [jax.device_put(a, sh) for a in concat_in]
    dev_zero = [jax.device_put(a, sh) for a in concat_zeros]
    for _ in range(warmup):
        out = sharded(*dev_in, *dev_zero)
    jax.block_until_ready(out)
    t0 = time.perf_counter()
    for _ in range(iters):
        out = sharded(*dev_in, *dev_zero)
    jax.block_until_ready(out)
    t1 = time.perf_counter()
    return (t1 - t0) / iters


# revision 27
# speedup vs baseline: 1.0860x; 1.0860x over previous
"""Trainium2 Bass kernel for nn_AttentionBlock (B=2, C=256, D=H=W=16).

Pipeline: GroupNorm(8) -> 1x1x1 conv QKV -> single-head attention over
N=4096 spatial tokens -> 1x1x1 conv proj -> residual.

Sharding: 8 cores = 2 batches x 4 query-chunks of 1024 tokens.  Each core
computes group-norm stats + K/V' for its full batch (redundantly across the
4 cores sharing a batch) and attention only for its 1024 queries.  Tokens
are permuted per-core (host side) so the core's own queries are tokens
0..1023 of its xb buffer; attention is permutation-invariant over keys, so
K/V/stats over the permuted order are exact.

Changes vs the 108us baseline (measured 72.2us, rel err 2.7e-3):
  - x shipped once as bf16 [2 halves, 128, 4096] (2 MB vs 5 MB of f32
    xb+xq): stats, K/V/Q matmuls, and the residual all read it; weights
    shipped bf16.  Tokens permuted per-core so own queries are tokens
    0..1023 (kills the separate xq load).
  - attention runs in fp8e4m3 with DoubleRow (256-deep contraction in one
    matmul at 0.5 cyc/row): Q/K packed [128, 2(half), n], V'^T packed as
    j-pair tiles [128, 16, 2, 272] (256 ch | ones | pad to a 16-aligned
    stride); probabilities exp(s-4) written fp8 (shift keeps exp under the
    448 fp8 max and cancels in the softmax normalize).  The 1/sqrt(C)
    score scale is split 1/4 into each of wq, wk so fp8 q/k values sit in
    e4m3's comfortable range.
  - group stats: DVE bn_stats over all 8 chunks per half (during the DMA),
    then one batched [128,2]-wide chain on DVE; the 32-channel group
    reduction uses the 32x32 block-transpose trick.
  - two-stage software pipeline across in-NEFF reps (bufs=2 tile pools):
    stage_prep (loads + stats + folds; no PE/ACT work) for rep k+1 is
    emitted mid-attention of rep k, so the reps-slope steady state hides
    the whole prologue.  Loads ride the ACT HWDGE queue and stores the SP
    HWDGE queue: a DMA trigger blocks its issuing sequencer until data is
    ready, so loads and stores must not share a queue with each other.

Layouts: scores are computed transposed (keys on partitions, queries on the
free axis) so the probability tiles are directly usable as the stationary
operand of the out-matmul; a ones-column in V'^T accumulates the softmax
denominator; outputs arrive [query, channel] and are transposed
back 128x128 via the PE transpose path.
"""

import os
import sys

import numpy as np

if "/opt/trn_rl_repo" not in sys.path:
    sys.path.insert(0, "/opt/trn_rl_repo")

import concourse.bass as bass
import concourse.mybir as mybir
import concourse.tile as tile
from concourse.bass_utils import run_bass_kernel_spmd

F32 = mybir.dt.float32
F32R = mybir.dt.float32r
BF16 = mybir.dt.bfloat16
FP8 = mybir.dt.float8e4
I32 = mybir.dt.int32
AF = mybir.ActivationFunctionType
ALU = mybir.AluOpType

B = 2
C = 256
N = 4096          # D*H*W tokens
NQ = 1024         # queries per core
G = 8             # groupnorm groups
GS = C // G       # 32 channels per group
EPS = 1e-5
NCORES = 8

SPLIT = True  # apply split_waits (needed for walrus)

_WS_CTR = [0]


def split_waits(nc, cap=1):
    """walrus allows a single sync wait per instruction; move excess
    sync_info.on_wait entries onto same-engine NoOps inserted before."""
    for fn in nc.m.functions:
        for blk in fn.blocks:
            out = []
            changed = False
            for ins in blk.instructions:
                si = ins.sync_info
                waits = list(si.on_wait) if si is not None else []
                if len(waits) > cap:
                    for i in range(0, len(waits) - cap, cap):
                        nop = mybir.InstNoOp(
                            name=f"I-waitsplit-{_WS_CTR[0]}",
                            engine=ins.engine,
                            ins=[], outs=[],
                        )
                        nop.sync_info = mybir.SyncInfo(
                            on_wait=waits[i:i + cap], on_update=[]
                        )
                        _WS_CTR[0] += 1
                        out.append(nop)
                    ins.sync_info = mybir.SyncInfo(
                        on_wait=waits[len(waits) - cap:],
                        on_update=list(si.on_update),
                    )
                    changed = True
                out.append(ins)
            if changed:
                blk.instructions = out


def build_bass(reps=1):
    nc = bass.Bass(trn_type="TRN2")

    # ---- DRAM I/O ----
    xb_d = nc.dram_tensor("xb", [2, 128, N], BF16, kind="ExternalInput")
    # f32r consts: blockdiag(1/32) 128 | identity 128 | scb 8
    cstf_d = nc.dram_tensor("cstf", [128, 264], F32R, kind="ExternalInput")
    # bf16 consts: wqk0 512 | wpv0 256 | wqk1 512 | wpv1 256
    cstb_d = nc.dram_tensor("cstb", [128, 1536], BF16, kind="ExternalInput")
    out_d = nc.dram_tensor("out", [2, 128, NQ], F32, kind="ExternalOutput")

    with tile.TileContext(nc) as tc:
        with (
            tc.tile_pool(name="consts", bufs=2) as consts,
            tc.tile_pool(name="fixed", bufs=1) as fixed,
            tc.tile_pool(name="work", bufs=4) as work,
            tc.tile_pool(name="small", bufs=4) as small,
            tc.tile_pool(name="pmisc", bufs=4, space="PSUM") as pmisc,
            tc.tile_pool(name="psO", bufs=1, space="PSUM") as psO,
        ):
            fx = {}  # rep-0-only fixed tiles (exp table, int consts, ones)

            def stage_prep(rep):
                """Input DMAs (SP queue), DVE group stats + scale/shift chain,
                weight folds.  No PE, no ACT: safe to emit mid-attention of
                the previous rep."""
                S = {}
                cstf = consts.tile([128, 264], F32R, tag="cstf")
                nc.scalar.dma_start(out=cstf, in_=cstf_d[:])
                cstb = consts.tile([128, 1536], BF16, tag="cstb")
                nc.scalar.dma_start(out=cstb, in_=cstb_d[:])
                S["ident"] = cstf[:, 128:256]
                scb = cstf.bitcast(F32)[:, 256:264]
                S["scb"] = scb
                S["wqk"] = [cstb[:, 0:512], cstb[:, 768:1280]]
                S["wpv"] = [cstb[:, 512:768], cstb[:, 1280:1536]]

                if rep == 0:
                    # exp ACT table preload + int consts + ones (once)
                    wtab = fixed.tile([128, 1], F32, tag="wtab")
                    nc.vector.memset(wtab, 0.0)
                    nc.scalar.activation(out=wtab, in_=wtab, func=AF.Exp)
                    fx["magic"] = fixed.tile([128, 2, 1], I32, tag="magic", name="magic")
                    nc.vector.memset(fx["magic"], 0x5F3759DF)
                    fx["sh1"] = fixed.tile([128, 2, 1], I32, tag="sh1", name="sh1")
                    nc.vector.memset(fx["sh1"], 1)
                    fx["ones"] = fixed.tile([128, 2], F32, tag="ones", name="ones")
                    nc.vector.memset(fx["ones"][:, 0:1], 1.0)
                    nc.vector.memset(fx["ones"][:, 1:2], 0.0)
                    fx["negs"] = fixed.tile([128, 1], F32, tag="negs", name="negs")
                    nc.vector.memset(fx["negs"], -4.0)
                    # PE HAM warm-up while DMAs run (rep 0 only)
                    wps = pmisc.tile([128, 512], F32, tag="pmisc",
                                     name="warmps")
                    for w in range(8):
                        nc.tensor.matmul(
                            wps[:, 0:128], lhsT=S["ident"], rhs=S["ident"],
                            start=True, stop=True, skip_group_check=True,
                        )

                xb = [consts.tile([128, N], BF16, tag=f"xb{t}",
                                  name=f"xb{t}_{rep}") for t in range(2)]
                S["xb"] = xb
                sts = [small.tile([128, 8, 6], F32, tag=f"bnst{t}",
                                  name=f"bnst{t}_{rep}") for t in range(2)]
                for t in range(2):
                    xrc = xb[t].rearrange("p (s c) -> p s c", c=512)
                    for ch in range(4):
                        nc.scalar.dma_start(
                            out=xb[t][:, ch * 1024:(ch + 1) * 1024],
                            in_=xb_d[t, :, ch * 1024:(ch + 1) * 1024],
                        )
                        for i in (2 * ch, 2 * ch + 1):
                            nc.vector.bn_stats(
                                out=sts[t][:, i, :], in_=xrc[:, i, :]
                            )
                        if rep == 0:
                            wps2 = pmisc.tile([128, 512], F32, tag="pmisc",
                                              name=f"warm{t}_{ch}")
                            nc.tensor.matmul(
                                wps2[:, 0:128], lhsT=S["ident"],
                                rhs=S["ident"],
                                start=True, stop=True, skip_group_check=True,
                            )

                # batched stats chain, all on DVE, both halves at once
                MV = small.tile([128, 2, 2], F32, tag="mv")     # [E, V] per t
                for t in range(2):
                    nc.vector.bn_aggr(out=MV[:, t, :], in_=sts[t])
                sq = small.tile([128, 2, 1], F32, tag="sq")
                nc.vector.tensor_tensor(sq, MV[:, :, 0:1], MV[:, :, 0:1],
                                        op=ALU.mult)
                Pr = small.tile([128, 2, 2], F32, tag="pr")     # [E, M2]
                nc.vector.tensor_tensor(Pr[:, :, 1:2], MV[:, :, 1:2], sq,
                                        op=ALU.add)
                nc.vector.tensor_copy(Pr[:, :, 0:1], MV[:, :, 0:1])
                # group reduce: broadcast cols to 32-blocks, 32x32 block
                # transpose, free-axis reduce => per-group sums on DVE
                pp = small.tile([128, 4, GS], F32, tag="pp")
                Prf = Pr.rearrange("p a b -> p (a b)")
                nc.vector.tensor_copy(
                    pp, Prf[:, :, None].to_broadcast([128, 4, GS])
                )
                tr = small.tile([128, 4, GS], F32, tag="tr")
                nc.vector.transpose(tr.rearrange("p a b -> p (a b)"),
                                    pp.rearrange("p a b -> p (a b)"))
                red = small.tile([128, 4], F32, tag="red")
                nc.vector.reduce_sum(red, tr, axis=mybir.AxisListType.X)
                nc.vector.tensor_scalar(red, red, 1.0 / GS, None, ALU.mult)
                red2 = red.rearrange("p (a b) -> p a b", b=2)  # [Eg, M2g]
                gsq = small.tile([128, 2, 1], F32, tag="gsq")
                nc.vector.tensor_tensor(gsq, red2[:, :, 0:1], red2[:, :, 0:1],
                                        op=ALU.mult)
                gv = small.tile([128, 2, 1], F32, tag="gv")
                nc.vector.scalar_tensor_tensor(
                    gv, red2[:, :, 1:2], EPS, gsq, op0=ALU.add,
                    op1=ALU.subtract,
                )
                # rstd = rsqrt(gv): bit-trick seed + 2 Newton steps
                yb = small.tile([128, 2, 1], I32, tag="yb")
                nc.vector.tensor_tensor(yb, gv.bitcast(I32), fx["sh1"],
                                        op=ALU.logical_shift_right)
                nc.vector.tensor_tensor(yb, fx["magic"], yb, op=ALU.subtract)
                y = yb.bitcast(F32)
                t2 = small.tile([128, 2, 1], F32, tag="t2")
                for _ in range(2):
                    nc.vector.tensor_tensor(t2, y, y, op=ALU.mult)
                    nc.vector.tensor_tensor(t2, t2, gv, op=ALU.mult)
                    nc.vector.tensor_scalar(t2, t2, -0.5, 1.5,
                                            ALU.mult, ALU.add)
                    nc.vector.tensor_tensor(y, y, t2, op=ALU.mult)
                # scale = rstd * norm_w ; shift = norm_b - mean*scale
                sc = small.tile([128, 2, 1], F32, tag="sc")
                nwv = scb[:, 4:6].rearrange("p (a b) -> p a b", b=1)
                nbv = scb[:, 6:8].rearrange("p (a b) -> p a b", b=1)
                nc.vector.tensor_tensor(sc, y, nwv, op=ALU.mult)
                sh = small.tile([128, 2, 1], F32, tag="shv")
                nc.vector.tensor_tensor(sh, red2[:, :, 0:1], sc, op=ALU.mult)
                nc.vector.tensor_tensor(sh, nbv, sh, op=ALU.subtract)
                shb = small.tile([128, 2, 1], BF16, tag="shb")
                nc.vector.tensor_copy(shb, sh)
                S["shb"] = shb

                # fold scale into weights (bf16)
                S["wqk_s"] = []
                S["wpv_s"] = []
                for t in range(2):
                    ws = consts.tile([128, 512], BF16, tag=f"wqks{t}",
                                     name=f"wqks{t}_{rep}")
                    nc.vector.tensor_scalar_mul(ws, S["wqk"][t],
                                                sc[:, t:t + 1, 0])
                    S["wqk_s"].append(ws)
                    ws2 = consts.tile([128, 256], BF16, tag=f"wpvs{t}",
                                      name=f"wpvs{t}_{rep}")
                    nc.vector.tensor_scalar_mul(ws2, S["wpv"][t],
                                                sc[:, t:t + 1, 0])
                    S["wpv_s"].append(ws2)
                return S

            def stage_main(rep, S, midhook):
                """K/V/Q + attention + epilogue + stores (SP queue)."""
                xb = S["xb"]
                wqk_s, wpv_s = S["wqk_s"], S["wpv_s"]
                ident = S["ident"]
                scb = S["scb"]

                # K packed fp8 [128, 2(m), N] for DoubleRow scores
                Kp = consts.tile([128, 2, N], FP8, tag="Kp", name=f"Kp_{rep}")
                for m in range(2):
                    for ch in range(N // 512):
                        ps = pmisc.tile([128, 512], F32, tag="pmisc")
                        for t in range(2):
                            nc.tensor.matmul(
                                ps,
                                lhsT=wqk_s[t][:, (2 + m) * 128:(3 + m) * 128],
                                rhs=xb[t][:, ch * 512:(ch + 1) * 512],
                                start=(t == 0),
                                stop=(t == 1),
                            )
                        if (m * (N // 512) + ch) % 2 == 0:
                            nc.scalar.activation(
                                out=Kp[:, m, ch * 512:(ch + 1) * 512], in_=ps,
                                func=AF.Copy,
                            )
                        else:
                            nc.vector.tensor_copy(
                                Kp[:, m, ch * 512:(ch + 1) * 512], ps
                            )

                # V'^T j-pair tiles [jj][o][256 ch | ones | pad to 272], fp8
                VT = consts.tile([128, 16, 2, 272], FP8, tag="VT",
                                 name=f"VT_{rep}")
                nc.vector.memset(VT[:, :, :, 256:272], 0.0)
                nc.vector.tensor_copy(
                    VT[:, :, :, 256:257],
                    fx["ones"][:, None, None, 0:1].to_broadcast([128, 16, 2, 1]),
                )
                for j in range(32):
                    ps = pmisc.tile([128, 512], F32, tag="pmisc")
                    for t in range(2):
                        nc.tensor.matmul(
                            ps[:, 0:256],
                            lhsT=xb[t][:, j * 128:(j + 1) * 128],
                            rhs=wpv_s[t],
                            start=(t == 0),
                            stop=(t == 1),
                        )
                    nc.vector.tensor_copy(
                        VT[:, j // 2, j % 2, 0:256], ps[:, 0:256]
                    )

                # shift-induced biases
                qbias = []
                fbias = []
                for m in range(2):
                    ps = pmisc.tile([128, 512], F32, tag="pmisc")
                    for t in range(2):
                        nc.tensor.matmul(
                            ps[:, 0:1],
                            lhsT=S["wqk"][t][:, m * 128:(m + 1) * 128],
                            rhs=S["shb"][:, t, :],
                            start=(t == 0),
                            stop=(t == 1),
                        )
                    qb_m = small.tile([128, 1], F32, tag=f"qbias{m}",
                                      name=f"qbias{m}_{rep}")
                    nc.vector.tensor_add(qb_m, ps[:, 0:1], scb[:, m:m + 1])
                    qbias.append(qb_m)
                for m in range(2):
                    ps = pmisc.tile([128, 512], F32, tag="pmisc")
                    for t in range(2):
                        nc.tensor.matmul(
                            ps[:, 0:1],
                            lhsT=S["wpv"][t][:, m * 128:(m + 1) * 128],
                            rhs=S["shb"][:, t, :],
                            start=(t == 0),
                            stop=(t == 1),
                        )
                    fb_m = small.tile([128, 1], F32, tag=f"fbias{m}",
                                      name=f"fbias{m}_{rep}")
                    nc.vector.tensor_add(fb_m, ps[:, 0:1],
                                         scb[:, 2 + m:3 + m])
                    fbias.append(fb_m)

                # Q packed fp8 [128, 2(m), NQ] (own tokens are xb[:, 0:1024])
                Qp = consts.tile([128, 2, NQ], FP8, tag="Qp", name=f"Qp_{rep}")
                for m in range(2):
                    for ch in range(NQ // 512):
                        ps = pmisc.tile([128, 512], F32, tag="pmisc")
                        for t in range(2):
                            nc.tensor.matmul(
                                ps,
                                lhsT=wqk_s[t][:, m * 128:(m + 1) * 128],
                                rhs=xb[t][:, ch * 512:(ch + 1) * 512],
                                start=(t == 0),
                                stop=(t == 1),
                            )
                        nc.vector.tensor_scalar_add(
                            Qp[:, m, ch * 512:(ch + 1) * 512], ps, qbias[m]
                        )

                # attention (scores and out both fp8 DoubleRow, 256-deep)
                fin = [consts.tile([128, NQ], F32, tag=f"fin{m}",
                                   name=f"fin{m}_{rep}") for m in range(2)]
                pes = {}

                def emit_scores(qt, jj):
                    """Scores + exp for the j-pair (2*jj, 2*jj+1): two
                    DoubleRow matmuls, two exps into one packed pe2 tile."""
                    pe2 = work.tile([128, 2, 512], FP8, tag="pexp",
                                    name=f"pe{rep}_{qt}_{jj}")
                    for o in range(2):
                        j = 2 * jj + o
                        ss = pmisc.tile([128, 512], F32, tag="pmisc",
                                        name=f"ss{rep}_{qt}_{j}")
                        nc.tensor.matmul(
                            ss,
                            lhsT=Kp[:, :, j * 128:(j + 1) * 128],
                            rhs=Qp[:, :, qt * 512:(qt + 1) * 512],
                            start=True, stop=True,
                            perf_mode=mybir.MatmulPerfMode.DoubleRow,
                        )
                        # exp(s - 4): keeps exp values inside fp8e4m3 range
                        # (max score ~6.2 -> exp 473 > 448 would NaN); the
                        # constant shift cancels in the softmax normalize
                        nc.scalar.activation(out=pe2[:, o, :], in_=ss,
                                             func=AF.Exp, bias=fx["negs"])
                    return pe2

                ret = None
                for qt in range(NQ // 512):
                    po = [psO.tile([128, 272], F32, tag=f"po{qs}",
                                   name=f"po{rep}_{qt}_{qs}")
                          for qs in range(4)]
                    if qt == 0:
                        pes[(0, 0)] = emit_scores(0, 0)
                    for jj in range(16):
                        pe2 = pes.pop((qt, jj))
                        for qs in range(4):
                            nc.tensor.matmul(
                                po[qs],
                                lhsT=pe2[:, :, qs * 128:(qs + 1) * 128],
                                rhs=VT[:, jj, :, :],
                                start=(jj == 0),
                                stop=(jj == 15),
                                perf_mode=mybir.MatmulPerfMode.DoubleRow,
                            )
                        if jj + 1 < 16:
                            pes[(qt, jj + 1)] = emit_scores(qt, jj + 1)
                        elif qt == 0:
                            pes[(1, 0)] = emit_scores(1, 0)
                    if qt == 0 and midhook is not None:
                        # emit the next rep's prep stage here: its DMA/DVE
                        # work overlaps this rep's second attention tile
                        ret = midhook()
                    # normalize, transpose to [channel, query], bias, residual.
                    # All DVE normalizes first, then the PE transposes, so the
                    # PE is not gated on an interleaved per-qs DVE chain.
                    aos = []
                    for qs in range(4):
                        zr = small.tile([128, 1], F32, tag="zr")
                        nc.vector.reciprocal(zr, po[qs][:, 256:257])
                        ao = work.tile([128, 256], F32R, tag="ao")
                        nc.vector.tensor_scalar_mul(ao, po[qs][:, 0:256], zr)
                        aos.append(ao)
                    for qs in range(4):
                        col = (qt * 4 + qs) * 128
                        for m in range(2):
                            tp = pmisc.tile([128, 512], F32, tag="pmisc")
                            nc.tensor.transpose(
                                tp.bitcast(F32R)[:, 0:128],
                                aos[qs][:, m * 128:(m + 1) * 128], ident
                            )
                            nc.vector.tensor_scalar_add(
                                fin[m][:, col:col + 128],
                                tp.bitcast(F32R)[:, 0:128],
                                fbias[m],
                            )
                            nc.gpsimd.tensor_add(
                                fin[m][:, col:col + 128],
                                fin[m][:, col:col + 128],
                                xb[m][:, col:col + 128],
                            )
                        # store per 256-col half as soon as it's complete
                        # (SP HWDGE queue: its sequencer is idle, so the
                        # trigger's data-wait blocks nothing)
                        if qs % 2 == 1:
                            cs = (qt * 4 + qs - 1) * 128
                            for m in range(2):
                                nc.sync.dma_start(
                                    out=out_d[m, :, cs:cs + 256],
                                    in_=fin[m][:, cs:cs + 256],
                                )
                return ret

            S = stage_prep(0)
            for rep in range(reps):
                if rep + 1 < reps:
                    nextS = stage_main(
                        rep, S, midhook=lambda r=rep: stage_prep(r + 1))
                    S = nextS
                else:
                    stage_main(rep, S, midhook=None)

    if SPLIT:
        split_waits(nc)
    return nc


_CACHED = {}
_RUNNER = {}


def _get_nc(reps=1):
    if reps not in _CACHED:
        _CACHED[reps] = build_bass(reps)
    return _CACHED[reps]


def _get_runner(reps=1):
    """Cached jitted shard_map runner over 8 cores."""
    if reps in _RUNNER:
        return _RUNNER[reps]
    import jax
    from jax.experimental.shard_map import shard_map
    from jax.sharding import Mesh, PartitionSpec
    from concourse import bass2jax, mybir as mb
    from concourse.bass2jax import _bass_exec_p, install_neuronx_cc_hook

    nc = _get_nc(reps)
    install_neuronx_cc_hook()
    assert nc.dbg_addr is None
    partition_name = nc.partition_id_tensor.name if nc.partition_id_tensor else None

    in_names = []
    out_names = []
    out_avals = []
    zero_outs = []
    for alloc in nc.m.functions[0].allocations:
        if not isinstance(alloc, mb.MemoryLocationSet):
            continue
        name = alloc.memorylocations[0].name
        if alloc.kind == "ExternalInput":
            if name != partition_name:
                in_names.append(name)
        elif alloc.kind == "ExternalOutput":
            out_names.append(name)
            shape = tuple(alloc.tensor_shape)
            dtype = mb.dt.np(alloc.dtype)
            out_avals.append(jax.core.ShapedArray(shape, dtype))
            zero_outs.append(np.zeros(shape, dtype))
    n_params = len(in_names)
    all_in_names = in_names + out_names
    if partition_name is not None:
        all_in_names = all_in_names + [partition_name]

    def _body(*args):
        operands = list(args)
        if partition_name is not None:
            operands.append(bass2jax.partition_id_tensor())
        outs = _bass_exec_p.bind(
            *operands,
            out_avals=tuple(out_avals),
            in_names=tuple(all_in_names),
            out_names=tuple(out_names),
            lowering_input_output_aliases=(),
            sim_require_finite=True,
            sim_require_nnan=True,
            nc=nc,
        )
        return tuple(outs)

    devices = jax.devices()[:NCORES]
    mesh = Mesh(np.asarray(devices), ("core",))
    n_outs = len(out_names)
    sharded = jax.jit(
        shard_map(
            _body,
            mesh=mesh,
            in_specs=(PartitionSpec("core"),) * (n_params + n_outs),
            out_specs=(PartitionSpec("core"),) * n_outs,
            check_rep=False,
        ),
        keep_unused=True,
    )
    _RUNNER[reps] = (sharded, in_names, out_names, out_avals, zero_outs, mesh)
    return _RUNNER[reps]


def _concat_inputs(in_maps, in_names, zero_outs):
    concat_in = [
        np.concatenate([np.asarray(in_maps[c][name]) for c in range(NCORES)], axis=0)
        for name in in_names
    ]
    concat_zeros = [
        np.zeros((NCORES * z.shape[0], *z.shape[1:]), z.dtype) for z in zero_outs
    ]
    return concat_in, concat_zeros


def _run(in_maps):
    sharded, in_names, out_names, out_avals, zero_outs, mesh = _get_runner()
    concat_in, concat_zeros = _concat_inputs(in_maps, in_names, zero_outs)
    out_arrs = sharded(*concat_in, *concat_zeros)
    return [
        {
            name: np.asarray(out_arrs[i]).reshape(NCORES, *out_avals[i].shape)[c]
            for i, name in enumerate(out_names)
        }
        for c in range(NCORES)
    ]


def _to_bf16(a):
    import ml_dtypes
    return np.ascontiguousarray(
        np.asarray(a, dtype=np.float32).astype(ml_dtypes.bfloat16)
    )


def _host_prep(x, norm_w, norm_b, qkv_w, qkv_b, proj_w, proj_b):
    # split the 1/sqrt(C)=1/16 score scale as 1/4 into each of wq, wk so
    # the fp8-stored q and k values sit in e4m3's comfortable range
    wq = qkv_w[0:C] * np.float32(0.25)
    wk = qkv_w[C:2 * C] * np.float32(0.25)
    wv = qkv_w[2 * C:3 * C]
    wqkT = np.ascontiguousarray(
        np.concatenate([wq, wk], axis=0).T
    ).reshape(2, 128, 512)
    wpvT = np.ascontiguousarray((proj_w @ wv).T).reshape(2, 128, 256)
    qb = (qkv_b[0:C] * np.float32(0.25)).reshape(2, 128).T  # [128, 2]
    cb = (proj_w @ qkv_b[2 * C:3 * C] + proj_b).reshape(2, 128).T
    nw = norm_w.reshape(2, 128).T
    nb = norm_b.reshape(2, 128).T
    scb = np.concatenate([qb, cb, nw, nb], axis=1).astype(np.float32)
    ident = np.eye(128, dtype=np.float32)
    bd = np.kron(np.eye(4, dtype=np.float32),
                 np.full((GS, GS), 1.0 / GS, dtype=np.float32))
    cstf = np.ascontiguousarray(
        np.concatenate([bd, ident, scb], axis=1)
    ).astype(np.float32)
    cstb = _to_bf16(np.concatenate(
        [wqkT[0], wpvT[0], wqkT[1], wpvT[1]], axis=1
    ))

    xf = x.reshape(B, 2, 128, N)
    in_maps = []
    for core in range(NCORES):
        b, qi = divmod(core, NCORES // B)
        # permute tokens: own 1024 queries first
        perm = np.concatenate([
            np.arange(qi * NQ, (qi + 1) * NQ),
            np.arange(0, qi * NQ),
            np.arange((qi + 1) * NQ, N),
        ])
        in_maps.append(
            {
                "xb": _to_bf16(xf[b][:, :, perm]),
                "cstf": cstf,
                "cstb": cstb,
            }
        )
    return in_maps


def kernel(x, norm_w, norm_b, qkv_w, qkv_b, proj_w, proj_b):
    x = np.ascontiguousarray(np.asarray(x, dtype=np.float32))
    norm_w = np.asarray(norm_w, dtype=np.float32)
    norm_b = np.asarray(norm_b, dtype=np.float32)
    qkv_w = np.asarray(qkv_w, dtype=np.float32)
    qkv_b = np.asarray(qkv_b, dtype=np.float32)
    proj_w = np.asarray(proj_w, dtype=np.float32)
    proj_b = np.asarray(proj_b, dtype=np.float32)

    Bs, Cs = x.shape[0], x.shape[1]
    assert (Bs, Cs) == (B, C) and x.shape[2] * x.shape[3] * x.shape[4] == N

    in_maps = _host_prep(x, norm_w, norm_b, qkv_w, qkv_b, proj_w, proj_b)
    results = _run(in_maps)

    y = np.empty((B, C, N), dtype=np.float32)
    for core in range(NCORES):
        b, qi = divmod(core, NCORES // B)
        y[b, :, qi * NQ:(qi + 1) * NQ] = results[core]["out"].reshape(C, NQ)
    return y.reshape(x.shape)


def bench(in_maps, iters=50, warmup=3, reps=1):
    """Amortized per-execution device time."""
    import time
    import jax
    from jax.sharding import NamedSharding, PartitionSpec

    sharded, in_names, out_names, out_avals, zero_outs, mesh = _get_runner(reps)
    concat_in, concat_zeros = _concat_inputs(in_maps, in_names, zero_outs)
    sh = NamedSharding(mesh, PartitionSpec("core"))
    dev_in = [jax.device_put(a, sh) for a in concat_in]
    dev_zero = [jax.device_put(a, sh) for a in concat_zeros]
    for _ in range(warmup):
        out = sharded(*dev_in, *dev_zero)
    jax.block_until_ready(out)
    t0 = time.perf_counter()
    for _ in range(iters):
        out = sharded(*dev_in, *dev_zero)
    jax.block_until_ready(out)
    t1 = time.perf_counter()
    return (t1 - t0) / iters
